# revision 42
# baseline (speedup 1.0000x reference)
"""Distributed Trainium2 kernel for ArceeAttention (GQA + RoPE + causal attention).

Sharding: DP over batch (2 groups of 4 cores) x TP-4 over heads within each group.
Each core: 8 q heads + 2 kv heads, full sequence of its batch.

Structure (engine-dense rework of the baseline; 1452us -> ~1170us):
  - Inputs are pre-cast to bf16 on the HOST in shard_inputs (halves all
    weight/hidden HBM traffic; kills the on-device staging casts entirely).
  - A0: chunk-major QKV with N=512 moving operand; hidT via xbar transposes
    [512,128] read straight from the bf16 hidden input.  ALL A0 DMA rides the
    single sync HWDGE ring: any DMA concurrent with an in-flight xbar
    transpose is starved by the transpose's exclusive hold on the DMA engines
    (and transposes on TWO HWDGE rings concurrently corrupt data).  Weight
    loads are interleaved with the chunk-0 transposes in 8-tile batches.
    RoPE trig precomputed once (quartered scratch so SBUF stays disjoint from
    the weight region -- no WAR serialization); sin sign folded into invfreq
    (signed angle).  qT/kT/v round-trip via DRAM to fit SBUF.
  - A1: scores in [128, 2x512] PSUM supertiles, ONE exp per supertile,
    denominator accumulated on DVE in bf16 and folded by ones-matmuls;
    o-proj pieces of slab s-1 interleaved between QK and PV inside slab s's
    supertile loop (fills the QK->exp->PV latency, keeps PE dense and the
    HAM clock warm); po/pden/pbc share a 2-buf PSUM pool; o-proj evacs
    alternate ACT/DVE.  ReduceScatter per full slab for slabs 0-2 and in two
    halves for slab 3 so the kernel tail only waits for a 2MB collective.
"""
import sys
import numpy as np

for _p in ("/opt/trn_rl_repo",):
    if _p not in sys.path:
        sys.path.append(_p)

import ml_dtypes  # noqa: E402
from concourse import bass, bacc, tile, mybir  # noqa: E402
from concourse.bass_utils import run_bass_kernel_spmd  # noqa: E402

F32 = mybir.dt.float32
F32R = mybir.dt.float32r
BF16 = mybir.dt.bfloat16
I32 = mybir.dt.int32

ROPE_THETA = 10000.0
D = 128  # head dim

_NC_CACHE = {}


def build_nc(S=2048, HID=4096, NQ=8, NKV=2, rope_f32r=True):
    REP = NQ // NKV           # q heads per kv head (4)
    QC = NQ * D               # q cols per core (1024)
    KC = NKV * D              # k (or v) cols per core (256)
    NQK = NQ + NKV            # q+k col-tiles (10)
    NHT = HID // 128          # hidden-dim tiles (32)
    CHUNK = 512
    NCH = S // CHUNK          # 4
    SLAB = 512
    NSLAB = S // SLAB         # 4
    SKT = SLAB // 128         # 4
    NTT = S // 128            # 16 token tiles
    SCALE = float(D) ** -0.5
    RG = [[0, 1, 2, 3], [4, 5, 6, 7]]
    RDT = BF16

    MAGIC = 12582912.0        # 1.5 * 2**23: float32 round-to-nearest-int trick
    TWOPI = float(2.0 * np.pi)
    INV2PI = float(1.0 / TWOPI)
    HALFPI = float(np.pi / 2.0)

    nc = bacc.Bacc(None, target_bir_lowering=False)
    hidden = nc.declare_dram_parameter("hidden_states", [S, HID], BF16, isOutput=False)
    positions = nc.declare_dram_parameter("positions", [1, S], I32, isOutput=False)
    w_qkv = nc.declare_dram_parameter("w_qkv", [HID, QC + 2 * KC], BF16, isOutput=False)
    w_o = nc.declare_dram_parameter("w_o", [QC, HID], BF16, isOutput=False)
    invf = nc.declare_dram_parameter("invfreq128", [128, 1], F32, isOutput=False)
    masks = nc.declare_dram_parameter("masks", [128, SKT, SLAB], BF16, isOutput=False)
    swapm = nc.declare_dram_parameter("swapmat", [128, 128], RDT, isOutput=False)
    eye128 = nc.declare_dram_parameter("eye128", [128, 128], BF16, isOutput=False)
    # slabs 0..2: rows = this core's 128-token quarter of the slab;
    # slab 3: rows 0:64 = quarter of half-slab 3a, rows 64:128 = 3b.
    out = nc.declare_dram_parameter("out", [NSLAB, SLAB // 4, HID], F32,
                                    isOutput=True)

    Exp = mybir.ActivationFunctionType.Exp
    Sin = mybir.ActivationFunctionType.Sin
    mul_op = mybir.AluOpType.mult
    add_op = mybir.AluOpType.add

    with tile.TileContext(nc) as tc:
      with tc.tile_pool(name="dram", bufs=1, space="DRAM") as dram:
        qkT = dram.tile([NQK, 128, S], BF16, name="qkT", tag="qkT")
        v_dr = dram.tile([NTT, 128, KC], BF16, name="v_dr", tag="v_dr")
        # slabs 0..2: one RS over the full slab; slab 3: two half-slab RS so
        # the kernel tail only waits for a 2MB collective.
        parts = [dram.tile([SLAB, HID], BF16, name=f"part{i}", tag=f"part{i}")
                 for i in range(NSLAB - 1)]
        parts3 = [dram.tile([SLAB // 2, HID], BF16, name=f"part3{i}",
                            tag=f"part3{i}") for i in range(2)]
        rsouts = [dram.tile([SLAB // 4, HID], BF16, name=f"rsout{i}",
                            tag=f"rsout{i}")
                  for i in range(NSLAB - 1)]
        rsouts3 = [dram.tile([SLAB // 8, HID], BF16, name=f"rsout3{i}",
                             tag=f"rsout3{i}") for i in range(2)]

        with tc.tile_pool(name="const", bufs=1) as cpool:
            invf_sb = cpool.tile([128, 1], F32, name="invf", tag="invf")
            nc.sync.dma_start(invf_sb[:], invf[:])
            ones_col = cpool.tile([128, 1], BF16, name="ones_col", tag="ones_col")
            nc.vector.memset(ones_col[:], 1.0)
            ones_row = cpool.tile([1, 128], F32, name="ones_row", tag="ones_row")
            nc.vector.memset(ones_row[:], 1.0)
            swap_sb = cpool.tile([128, 128], RDT, name="swapm", tag="swapm")
            nc.sync.dma_start(swap_sb[:], swapm[:])
            eye_sb = cpool.tile([128, 128], BF16, name="eye", tag="eye")
            nc.sync.dma_start(eye_sb[:], eye128[:])
            cosc = cpool.tile([128, S], BF16, name="cosc", tag="cosc")
            sinc = cpool.tile([128, S], BF16, name="sinc", tag="sinc")

            # ================= A0: QKV + RoPE =================
            with (
                tc.tile_pool(name="wq", bufs=1) as wqp,
                tc.tile_pool(name="hidT", bufs=2) as hTp,
                tc.tile_pool(name="rope", bufs=3) as rp,
                tc.tile_pool(name="vw", bufs=2) as vwp,
                tc.tile_pool(name="hnat", bufs=1) as hnp,
                tc.tile_pool(name="psA", bufs=2, space="PSUM") as psA,
                tc.tile_pool(name="psW", bufs=2, space="PSUM") as psW,
                tc.tile_pool(name="psV", bufs=1, space="PSUM") as psV,
                tc.tile_pool(name="psH", bufs=2, space="PSUM") as psH,
            ):
                WCOLS = QC + 2 * KC
                wq_flat = wqp.tile([128, NHT, WCOLS], BF16, name="wq", tag="wq")
                wq_sb = [wq_flat[:, h, :] for h in range(NHT)]
                # All A0 DMA goes on the ONE sync HWDGE ring: any DMA that
                # runs concurrently with an in-flight xbar transpose is
                # starved by the transpose's exclusive hold on the DMA
                # engines, so serialize everything on one ring instead.
                # (Weight loads are interleaved with the chunk-0 transposes
                # in batches of 8 below.)

                # ---- trig precompute (quartered; small disjoint scratch) ----
                with (
                    tc.tile_pool(name="trig", bufs=1) as tgp,
                    tc.tile_pool(name="psTR", bufs=1, space="PSUM") as ptr,
                ):
                    for qi in range(S // 512):
                        q0 = 512 * qi
                        pos_i = tgp.tile([1, 512], I32, name="posi", tag="posi")
                        nc.sync.dma_start(pos_i[:], positions[0:1, q0:q0 + 512])
                        pos_c = tgp.tile([1, 512], F32, name="posc", tag="posc")
                        nc.vector.tensor_copy(pos_c[:], pos_i[:])
                        ppos = ptr.tile([128, 512], F32, name="ppos", tag="ppos")
                        nc.tensor.matmul(ppos[:], ones_row[:], pos_c[:],
                                         start=True, stop=True)
                        # signed angle: invf rows 0..63 negative -> sin gets
                        # the sign for free, cos is even so unaffected.
                        ang = tgp.tile([128, 512], F32, name="ang", tag="ang")
                        nc.vector.tensor_scalar_mul(ang[:], ppos[:], invf_sb[:])
                        tmp = tgp.tile([128, 512], F32, name="ttmp", tag="ttmp")
                        red = tgp.tile([128, 512], F32, name="tred", tag="tred")
                        for dst, phase in ((cosc, HALFPI), (sinc, 0.0)):
                            nc.vector.tensor_scalar(
                                tmp[:], ang[:], INV2PI, phase * INV2PI,
                                op0=mul_op, op1=add_op)
                            nc.vector.tensor_scalar_add(tmp[:], tmp[:], MAGIC)
                            nc.vector.tensor_scalar_sub(tmp[:], tmp[:], MAGIC)
                            nc.vector.scalar_tensor_tensor(
                                red[:], tmp[:], -TWOPI, ang[:],
                                op0=mul_op, op1=add_op)
                            if phase != 0.0:
                                nc.vector.tensor_scalar_add(red[:], red[:],
                                                            phase)
                            nc.vector.tensor_scalar_min(red[:], red[:],
                                                        3.141592)
                            nc.vector.tensor_scalar_max(red[:], red[:],
                                                        -3.141592)
                            nc.scalar.activation(dst[:, q0:q0 + 512], red[:],
                                                 Sin)

                # preload the Exp table set while A0 runs (saves the ~2.7us
                # table switch from the A1 critical path)
                dume = rp.tile([1, 2], F32, name="dume", tag="dume")
                nc.vector.memset(dume[:], 0.0)
                nc.scalar.activation(dume[:], dume[:], Exp)

                for c in range(NCH):
                    c0 = CHUNK * c
                    hidT = [hTp.tile([128, CHUNK], BF16,
                                     name=f"hidT{h}", tag=f"hidT{h}")
                            for h in range(NHT)]
                    if c == 0:
                        # Chunk 0: transpose on the (otherwise idle) PE via
                        # transpose-mode matmuls. This halves the serial sync
                        # ring work at the head (weights + natural loads only)
                        # and fills the PE while weights stream in.
                        for tt in range(CHUNK // 128):
                            hnA = hnp.tile([128, HID // 2], BF16, name="hnA",
                                           tag="hnA")
                            hnB = hnp.tile([128, HID // 2], BF16, name="hnB",
                                           tag="hnB")
                            nc.sync.dma_start(
                                hnA[:], hidden[128 * tt:128 * (tt + 1),
                                               0:HID // 2])
                            nc.sync.dma_start(
                                hnB[:], hidden[128 * tt:128 * (tt + 1),
                                               HID // 2:HID])
                            for h in range(NHT):
                                if tt == 0 and h % 4 == 0:
                                    for hh in range(h, h + 4):
                                        nc.sync.dma_start(
                                            wq_flat[:, hh, :],
                                            w_qkv[128 * hh:128 * (hh + 1), :])
                                src_t = (hnA if h < NHT // 2 else hnB)
                                hcol = h % (NHT // 2)
                                psT = psH.tile([128, 128], BF16, name="psT",
                                               tag="psT")
                                nc.tensor.transpose(
                                    psT[:],
                                    src_t[:, 128 * hcol:128 * (hcol + 1)],
                                    eye_sb[:])
                                nc.scalar.copy(
                                    hidT[h][:, 128 * tt:128 * (tt + 1)],
                                    psT[:])
                    else:
                        for h in range(NHT):
                            nc.sync.dma_start_transpose(
                                hidT[h][:], hidden[c0:c0 + CHUNK,
                                                   128 * h:128 * (h + 1)])
                    # last chunk: emit k cts first so the A1 kT/qT loads can
                    # start before the rest of the chunk finishes
                    ct_order = ([NQ, NQ + 1] + list(range(NQ))
                                if c == NCH - 1 else list(range(NQK)))
                    for ct in ct_order:
                        pq = psA.tile([128, CHUNK], F32, name="pq", tag="pq")
                        for h in range(NHT):
                            nc.tensor.matmul(
                                pq[:],
                                wq_sb[h][:, 128 * ct:128 * (ct + 1)],
                                hidT[h][:],
                                start=(h == 0), stop=(h == NHT - 1),
                            )
                        qw = rp.tile([128, CHUNK], RDT, name="qw", tag="qw")
                        nc.scalar.copy(qw[:], pq[:])
                        pswap = psW.tile([128, CHUNK], F32, name="pswap",
                                         tag="pswap")
                        nc.tensor.matmul(pswap[:], swap_sb[:], qw[:],
                                         start=True, stop=True)
                        qcos = rp.tile([128, CHUNK], BF16, name="qcos",
                                       tag="qcos")
                        nc.vector.tensor_mul(qcos[:], qw[:],
                                             cosc[:, c0:c0 + CHUNK])
                        rot = rp.tile([128, CHUNK], BF16, name="rot", tag="rot")
                        nc.vector.tensor_mul(rot[:], pswap[:],
                                             sinc[:, c0:c0 + CHUNK])
                        qout = rp.tile([128, CHUNK], BF16, name="qout",
                                       tag="qout")
                        nc.vector.tensor_add(qout[:], qcos[:], rot[:])
                        nc.sync.dma_start(qkT[ct][:, c0:c0 + CHUNK], qout[:])
                    for tt in range(CHUNK // 128):
                        pv = psV.tile([128, KC], F32, name="pv", tag="pv")
                        for h in range(NHT):
                            nc.tensor.matmul(
                                pv[:],
                                hidT[h][:, 128 * tt:128 * (tt + 1)],
                                wq_sb[h][:, QC + KC:QC + 2 * KC],
                                start=(h == 0), stop=(h == NHT - 1),
                            )
                        vw = vwp.tile([128, KC], BF16, name="vw", tag="vw")
                        nc.scalar.copy(vw[:], pv[:])
                        nc.sync.dma_start(v_dr[c * (CHUNK // 128) + tt], vw[:])

            # ============ A1: attention + o-proj + RS ============
            with (
                tc.tile_pool(name="qT", bufs=1) as qTp,
                tc.tile_pool(name="kT", bufs=1) as kTp,
                tc.tile_pool(name="vsb", bufs=1) as vp,
                tc.tile_pool(name="wo", bufs=1) as wop,
                tc.tile_pool(name="maskp", bufs=1) as mkp,
                tc.tile_pool(name="at", bufs=2) as atp,
                tc.tile_pool(name="den", bufs=2) as dnp,
                tc.tile_pool(name="pt", bufs=3) as ptp,
                tc.tile_pool(name="bcp", bufs=2) as bcp,
                tc.tile_pool(name="ot", bufs=3) as otp,
                tc.tile_pool(name="psS", bufs=2, space="PSUM") as psS,
                tc.tile_pool(name="psPV", bufs=2, space="PSUM") as psPV,
                tc.tile_pool(name="psX", bufs=2, space="PSUM") as psX,
            ):
                psO = psX
                kT_sb = [kTp.tile([128, S], BF16, name=f"kT{i}", tag=f"kT{i}")
                         for i in range(NKV)]
                v_flat = vp.tile([128, NTT, KC], BF16, name="vfl", tag="vfl")
                v_sb = [v_flat[:, t, :] for t in range(NTT)]
                qT_sb = [qTp.tile([128, S], BF16, name=f"qT{i}", tag=f"qT{i}")
                         for i in range(NQ)]
                mask_sb = mkp.tile([128, SKT, SLAB], BF16, name="masks",
                                   tag="masks")
                wo_flat = wop.tile([128, NQ, HID], BF16, name="wofl", tag="wofl")
                wo_sb = [wo_flat[:, q, :] for q in range(NQ)]
                # load order tuned so slab-0/head-0 can start ASAP
                for i in range(NKV):
                    nc.sync.dma_start(kT_sb[i][:], qkT[NQ + i])
                nc.sync.dma_start(qT_sb[0][:], qkT[0])
                nc.sync.dma_start(mask_sb[:], masks[:])
                for t in range(SKT):
                    nc.sync.dma_start(v_flat[:, t, :], v_dr[t])
                for i in range(1, NQ):
                    nc.sync.dma_start(qT_sb[i][:], qkT[i])
                for t in range(SKT, NTT):
                    nc.sync.dma_start(v_flat[:, t, :], v_dr[t])
                for q in range(NQ):
                    nc.sync.dma_start(wo_flat[:, q, :],
                                      w_o[128 * q:128 * (q + 1), :])

                def oproj_piece(s, pi, ats):
                    tt, hc = divmod(pi, HID // 512)
                    po = psO.tile([128, 512], F32, name="po", tag="px")
                    for q in range(NQ):
                        nc.tensor.matmul(
                            po[:],
                            ats[q][:, 128 * tt:128 * (tt + 1)],
                            wo_sb[q][:, 512 * hc:512 * (hc + 1)],
                            start=(q == 0), stop=(q == NQ - 1),
                        )
                    ot = otp.tile([128, 512], BF16, name="ot", tag="ot")
                    if pi % 2 == 0:
                        nc.scalar.copy(ot[:], po[:])
                    else:
                        nc.vector.tensor_copy(ot[:], po[:])
                    if s < NSLAB - 1:
                        nc.sync.dma_start(
                            parts[s][128 * tt:128 * (tt + 1),
                                     512 * hc:512 * (hc + 1)], ot[:])
                        if pi == 4 * NQ - 1:
                            nc.gpsimd.collective_compute(
                                "ReduceScatter", mybir.AluOpType.add,
                                replica_groups=RG,
                                ins=[parts[s].opt()], outs=[rsouts[s].opt()])
                            nc.gpsimd.dma_start(out[s], rsouts[s][:])
                    else:
                        half, tt2 = divmod(tt, 2)
                        nc.sync.dma_start(
                            parts3[half][128 * tt2:128 * (tt2 + 1),
                                         512 * hc:512 * (hc + 1)], ot[:])
                        if pi in (2 * NQ - 1, 4 * NQ - 1):
                            half = pi // (2 * NQ)
                            nc.gpsimd.collective_compute(
                                "ReduceScatter", mybir.AluOpType.add,
                                replica_groups=RG,
                                ins=[parts3[half].opt()],
                                outs=[rsouts3[half].opt()])
                            nc.gpsimd.dma_start(
                                out[NSLAB - 1][64 * half:64 * (half + 1), :],
                                rsouts3[half][:])

                at_prev = None
                for s in range(NSLAB):
                    s0 = SLAB * s
                    NSUP = 2 * (s + 1)
                    sup_total = NQ * NSUP
                    sup_done = 0
                    pieces_emitted = 0
                    at_cur = []
                    for hq in range(NQ):
                        kvh = hq // REP
                        ppv = psPV.tile([128, SLAB], F32, name="ppv", tag="ppv")
                        den = dnp.tile([128, 2, 512], BF16, name="den",
                                       tag="den")
                        den_src = None
                        for j in range(NSUP):
                            ps = psS.tile([128, 2, 512], F32, name="ps",
                                          tag="ps")
                            for u in range(2):
                                kt = 2 * j + u
                                nc.tensor.matmul(
                                    ps[:, u, :],
                                    kT_sb[kvh][:, 128 * kt:128 * (kt + 1)],
                                    qT_sb[hq][:, s0:s0 + SLAB],
                                    start=True, stop=True,
                                )
                            pt = ptp.tile([128, 2, 512], BF16, name="pt",
                                          tag="pt")
                            nc.scalar.activation(pt[:], ps[:], Exp, scale=SCALE)
                            dj = j - 2 * s
                            if 0 <= dj <= 1:
                                nc.vector.tensor_mul(
                                    pt[:], pt[:],
                                    mask_sb[:, 2 * dj:2 * dj + 2, :])
                            if NSUP > 1:
                                if j == 0:
                                    nc.vector.tensor_copy(den[:], pt[:])
                                else:
                                    nc.vector.tensor_add(den[:], den[:], pt[:])
                                den_src = den
                            else:
                                den_src = pt
                            # fill the QK->exp->PV latency with o-proj MMs of
                            # the previous slab (keeps the PE dense and warm)
                            sup_done += 1
                            if at_prev is not None:
                                due = (4 * NQ * sup_done) // sup_total
                                while pieces_emitted < due:
                                    oproj_piece(s - 1, pieces_emitted, at_prev)
                                    pieces_emitted += 1
                            for u in range(2):
                                kt = 2 * j + u
                                nc.tensor.matmul(
                                    ppv[:],
                                    v_sb[kt][:, D * kvh:D * (kvh + 1)],
                                    pt[:, u, :],
                                    start=(j == 0 and u == 0),
                                    stop=(j == NSUP - 1 and u == 1),
                                )
                        pden = psX.tile([128, 512], F32, name="pden",
                                        tag="px")
                        for u in range(2):
                            nc.tensor.matmul(pden[0:1, :], ones_col[:],
                                             den_src[:, u, :],
                                             start=(u == 0), stop=(u == 1))
                        rec = bcp.tile([1, 512], F32, name="rec", tag="rec")
                        nc.vector.reciprocal_approx_fast(rec[:], pden[0:1, :])
                        pbc = psX.tile([128, 512], F32, name="pbc", tag="px")
                        nc.tensor.matmul(pbc[:], ones_row[:], rec[:],
                                         start=True, stop=True)
                        bc = bcp.tile([128, 512], F32, name="bc", tag="bc")
                        nc.vector.tensor_copy(bc[:], pbc[:])
                        at = atp.tile([128, SLAB], BF16,
                                      name=f"at{hq}", tag=f"at{hq}")
                        nc.vector.tensor_mul(at[:], ppv[:], bc[:])
                        at_cur.append(at)
                    while at_prev is not None and pieces_emitted < 4 * NQ:
                        oproj_piece(s - 1, pieces_emitted, at_prev)
                        pieces_emitted += 1
                    at_prev = at_cur
                for pi in range(4 * NQ):
                    oproj_piece(NSLAB - 1, pi, at_prev)

    nc.compile()
    return nc


def make_consts(S=2048):
    SLAB = min(512, S)
    SKT = SLAB // 128
    d_half = np.arange(0, D, 2, dtype=np.float32) / D
    invfreq = (1.0 / (ROPE_THETA ** d_half)).astype(np.float32)  # [64]
    # signed: rows 0..63 negative (sin sign trick), cos unaffected (even fn)
    invf128 = np.concatenate([-invfreq, invfreq]).reshape(128, 1).astype(np.float32)
    p = np.arange(128).reshape(128, 1, 1)
    j = np.arange(SKT).reshape(1, SKT, 1)
    q = np.arange(SLAB).reshape(1, 1, SLAB)
    masks = ((j * 128 + p) <= q).astype(ml_dtypes.bfloat16)  # [128, SKT, SLAB]
    swapmat = np.zeros((128, 128), np.float32)
    for pp in range(128):
        swapmat[pp, (pp + 64) % 128] = 1.0
    eye = np.eye(128, dtype=np.float32).astype(ml_dtypes.bfloat16)
    return invf128, masks, swapmat.astype(ml_dtypes.bfloat16), eye


def shard_inputs(hidden_states, positions, w_qkv, w_o, n_q_total=32, n_kv_total=8,
                 tp=4):
    """Returns in_maps for 8 cores: DP over batch x TP over heads."""
    B, S, HIDDEN = hidden_states.shape
    q_size = n_q_total * D
    kv_size = n_kv_total * D
    nq = n_q_total // tp           # q heads per core
    nkv = n_kv_total // tp         # kv heads per core
    invf128, masks, swapmat, eye = make_consts(S=S)
    in_maps = []
    for c in range(8):
        g, r = divmod(c, tp)
        wq = w_qkv[:, nq * D * r: nq * D * (r + 1)]
        wk = w_qkv[:, q_size + nkv * D * r: q_size + nkv * D * (r + 1)]
        wv = w_qkv[:, q_size + kv_size + nkv * D * r: q_size + kv_size + nkv * D * (r + 1)]
        in_maps.append({
            "hidden_states": np.ascontiguousarray(
                hidden_states[g]).astype(ml_dtypes.bfloat16),
            "positions": np.ascontiguousarray(positions[g:g + 1]).astype(np.int32),
            "w_qkv": np.ascontiguousarray(
                np.concatenate([wq, wk, wv], axis=1)).astype(ml_dtypes.bfloat16),
            "w_o": np.ascontiguousarray(
                w_o[nq * D * r: nq * D * (r + 1), :]).astype(ml_dtypes.bfloat16),
            "invfreq128": invf128,
            "masks": masks,
            "swapmat": swapmat,
            "eye128": eye,
        })
    return in_maps


def assemble_output(results, B=2, S=2048, HIDDEN=4096, tp=4):
    SLAB = min(512, S)
    NSLAB = S // SLAB
    out = np.empty((B, S, HIDDEN), dtype=np.float32)
    for c in range(8):
        g, r = divmod(c, tp)
        o = np.asarray(results[c]["out"]).reshape(NSLAB, SLAB // 4, HIDDEN)
        for s in range(NSLAB - 1):
            t0 = SLAB * s + 128 * r
            out[g, t0:t0 + 128, :] = o[s]
        for half in range(2):
            t0 = SLAB * (NSLAB - 1) + 256 * half + 64 * r
            out[g, t0:t0 + 64, :] = o[NSLAB - 1][64 * half:64 * (half + 1)]
    return out


def kernel(hidden_states, positions, w_qkv, w_o):
    hidden_states = np.asarray(hidden_states, dtype=np.float32)
    positions = np.asarray(positions, dtype=np.int32)
    w_qkv = np.asarray(w_qkv, dtype=np.float32)
    w_o = np.asarray(w_o, dtype=np.float32)
    B, S, HIDDEN = hidden_states.shape

    key = (S, HIDDEN)
    if key not in _NC_CACHE:
        try:
            _NC_CACHE[key] = build_nc(S=S, HID=HIDDEN, rope_f32r=True)
        except Exception:
            _NC_CACHE[key] = build_nc(S=S, HID=HIDDEN, rope_f32r=False)
    nc = _NC_CACHE[key]

    in_maps = shard_inputs(hidden_states, positions, w_qkv, w_o)
    res = run_bass_kernel_spmd(nc, in_maps, core_ids=list(range(8)))
    return assemble_output(res.results, B=B, S=S, HIDDEN=HIDDEN)


if __name__ == "__main__":
    rng = np.random.default_rng(0)
    B, S, HIDDEN = 2, 2048, 4096
    hs = rng.standard_normal((B, S, HIDDEN), dtype=np.float32)
    pos = np.arange(B * S, dtype=np.int32).reshape(B, S)
    wq = rng.standard_normal((HIDDEN, 6144), dtype=np.float32) * HIDDEN ** -0.5
    wo = rng.standard_normal((4096, HIDDEN), dtype=np.float32) * 4096 ** -0.5
    o = kernel(hs, pos, wq, wo)
    print(o.shape, o.dtype)


# revision 43
# speedup vs baseline: 1.0689x; 1.0689x over previous
"""Distributed Trainium2 kernel for ArceeAttention (GQA + RoPE + causal attention).

Sharding: DP over batch (2 groups of 4 cores) x TP-4 over heads within each group.
Each core: 8 q heads + 2 kv heads, full sequence of its batch.

Structure (engine-dense rework of the baseline; 1452us -> ~1170us):
  - Inputs are pre-cast to bf16 on the HOST in shard_inputs (halves all
    weight/hidden HBM traffic; kills the on-device staging casts entirely).
  - A0: chunk-major QKV with N=512 moving operand; hidT via xbar transposes
    [512,128] read straight from the bf16 hidden input.  ALL A0 DMA rides the
    single sync HWDGE ring: any DMA concurrent with an in-flight xbar
    transpose is starved by the transpose's exclusive hold on the DMA engines
    (and transposes on TWO HWDGE rings concurrently corrupt data).  Weight
    loads are interleaved with the chunk-0 transposes in 8-tile batches.
    RoPE trig precomputed once (quartered scratch so SBUF stays disjoint from
    the weight region -- no WAR serialization); sin sign folded into invfreq
    (signed angle).  qT/kT/v round-trip via DRAM to fit SBUF.
  - A1: scores in [128, 2x512] PSUM supertiles, ONE exp per supertile,
    denominator accumulated on DVE in bf16 and folded by ones-matmuls;
    o-proj pieces of slab s-1 interleaved between QK and PV inside slab s's
    supertile loop (fills the QK->exp->PV latency, keeps PE dense and the
    HAM clock warm); po/pden/pbc share a 2-buf PSUM pool; o-proj evacs
    alternate ACT/DVE.  ReduceScatter per full slab for slabs 0-2 and in two
    halves for slab 3 so the kernel tail only waits for a 2MB collective.
"""
import sys
import numpy as np

for _p in ("/opt/trn_rl_repo",):
    if _p not in sys.path:
        sys.path.append(_p)

import ml_dtypes  # noqa: E402
from concourse import bass, bacc, tile, mybir  # noqa: E402
from concourse.bass_utils import run_bass_kernel_spmd  # noqa: E402

F32 = mybir.dt.float32
F32R = mybir.dt.float32r
BF16 = mybir.dt.bfloat16
I32 = mybir.dt.int32

ROPE_THETA = 10000.0
D = 128  # head dim

_NC_CACHE = {}


def build_nc(S=2048, HID=4096, NQ=8, NKV=2, rope_f32r=True):
    REP = NQ // NKV           # q heads per kv head (4)
    QC = NQ * D               # q cols per core (1024)
    KC = NKV * D              # k (or v) cols per core (256)
    NQK = NQ + NKV            # q+k col-tiles (10)
    NHT = HID // 128          # hidden-dim tiles (32)
    CHUNK = 512
    NCH = S // CHUNK          # 4
    SLAB = 512
    NSLAB = S // SLAB         # 4
    SKT = SLAB // 128         # 4
    NTT = S // 128            # 16 token tiles
    SCALE = float(D) ** -0.5
    RG = [[0, 1, 2, 3], [4, 5, 6, 7]]
    RDT = BF16

    MAGIC = 12582912.0        # 1.5 * 2**23: float32 round-to-nearest-int trick
    TWOPI = float(2.0 * np.pi)
    INV2PI = float(1.0 / TWOPI)
    HALFPI = float(np.pi / 2.0)

    nc = bacc.Bacc(None, target_bir_lowering=False)
    hidden = nc.declare_dram_parameter("hidden_states", [S, HID], BF16, isOutput=False)
    positions = nc.declare_dram_parameter("positions", [1, S], I32, isOutput=False)
    w_qkv = nc.declare_dram_parameter("w_qkv", [HID, QC + 2 * KC], BF16, isOutput=False)
    w_o = nc.declare_dram_parameter("w_o", [QC, HID], BF16, isOutput=False)
    invf = nc.declare_dram_parameter("invfreq128", [128, 1], F32, isOutput=False)
    masks = nc.declare_dram_parameter("masks", [128, SKT, SLAB], BF16, isOutput=False)
    swapm = nc.declare_dram_parameter("swapmat", [128, 128], RDT, isOutput=False)
    # slabs 0..2: rows = this core's 128-token quarter of the slab;
    # slab 3: rows 0:64 = quarter of half-slab 3a, rows 64:128 = 3b.
    out = nc.declare_dram_parameter("out", [NSLAB, SLAB // 4, HID], F32,
                                    isOutput=True)

    Exp = mybir.ActivationFunctionType.Exp
    Sin = mybir.ActivationFunctionType.Sin
    mul_op = mybir.AluOpType.mult
    add_op = mybir.AluOpType.add

    with tile.TileContext(nc) as tc:
      with tc.tile_pool(name="dram", bufs=1, space="DRAM") as dram:
        qkT = dram.tile([NQK, 128, S], BF16, name="qkT", tag="qkT")
        v_dr = dram.tile([NTT, 128, KC], BF16, name="v_dr", tag="v_dr")
        # slabs 0..2: one RS over the full slab; slab 3: two half-slab RS so
        # the kernel tail only waits for a 2MB collective.
        parts = [dram.tile([SLAB, HID], BF16, name=f"part{i}", tag=f"part{i}")
                 for i in range(NSLAB - 1)]
        parts3 = [dram.tile([SLAB // 2, HID], BF16, name=f"part3{i}",
                            tag=f"part3{i}") for i in range(2)]
        rsouts = [dram.tile([SLAB // 4, HID], BF16, name=f"rsout{i}",
                            tag=f"rsout{i}")
                  for i in range(NSLAB - 1)]
        rsouts3 = [dram.tile([SLAB // 8, HID], BF16, name=f"rsout3{i}",
                             tag=f"rsout3{i}") for i in range(2)]

        with tc.tile_pool(name="const", bufs=1) as cpool:
            invf_sb = cpool.tile([128, 1], F32, name="invf", tag="invf")
            nc.sync.dma_start(invf_sb[:], invf[:])
            ones_col = cpool.tile([128, 1], BF16, name="ones_col", tag="ones_col")
            nc.vector.memset(ones_col[:], 1.0)
            ones_row = cpool.tile([1, 128], F32, name="ones_row", tag="ones_row")
            nc.vector.memset(ones_row[:], 1.0)
            swap_sb = cpool.tile([128, 128], RDT, name="swapm", tag="swapm")
            nc.sync.dma_start(swap_sb[:], swapm[:])
            cosc = cpool.tile([128, S], BF16, name="cosc", tag="cosc")
            sinc = cpool.tile([128, S], BF16, name="sinc", tag="sinc")

            # ================= A0: QKV + RoPE =================
            with (
                tc.tile_pool(name="wq", bufs=1) as wqp,
                tc.tile_pool(name="hidT", bufs=2) as hTp,
                tc.tile_pool(name="rope", bufs=3) as rp,
                tc.tile_pool(name="vw", bufs=2) as vwp,
                tc.tile_pool(name="psA", bufs=3, space="PSUM") as psA,
                tc.tile_pool(name="psW", bufs=2, space="PSUM") as psW,
                tc.tile_pool(name="psV", bufs=2, space="PSUM") as psV,
            ):
                WCOLS = QC + 2 * KC
                wq_flat = wqp.tile([128, NHT, WCOLS], BF16, name="wq", tag="wq")
                wq_sb = [wq_flat[:, h, :] for h in range(NHT)]
                # All A0 DMA goes on the ONE sync HWDGE ring: any DMA that
                # runs concurrently with an in-flight xbar transpose is
                # starved by the transpose's exclusive hold on the DMA
                # engines, so serialize everything on one ring instead.
                # (Weight loads are interleaved with the chunk-0 transposes
                # in batches of 8 below.)

                # ---- trig precompute (quartered; small disjoint scratch) ----
                with (
                    tc.tile_pool(name="trig", bufs=1) as tgp,
                    tc.tile_pool(name="psTR", bufs=1, space="PSUM") as ptr,
                ):
                    for qi in range(S // 512):
                        q0 = 512 * qi
                        pos_i = tgp.tile([1, 512], I32, name="posi", tag="posi")
                        nc.sync.dma_start(pos_i[:], positions[0:1, q0:q0 + 512])
                        pos_c = tgp.tile([1, 512], F32, name="posc", tag="posc")
                        nc.vector.tensor_copy(pos_c[:], pos_i[:])
                        ppos = ptr.tile([128, 512], F32, name="ppos", tag="ppos")
                        nc.tensor.matmul(ppos[:], ones_row[:], pos_c[:],
                                         start=True, stop=True)
                        # signed angle: invf rows 0..63 negative -> sin gets
                        # the sign for free, cos is even so unaffected.
                        ang = tgp.tile([128, 512], F32, name="ang", tag="ang")
                        nc.vector.tensor_scalar_mul(ang[:], ppos[:], invf_sb[:])
                        tmp = tgp.tile([128, 512], F32, name="ttmp", tag="ttmp")
                        red = tgp.tile([128, 512], F32, name="tred", tag="tred")
                        for dst, phase in ((cosc, HALFPI), (sinc, 0.0)):
                            nc.vector.tensor_scalar(
                                tmp[:], ang[:], INV2PI, phase * INV2PI,
                                op0=mul_op, op1=add_op)
                            nc.vector.tensor_scalar_add(tmp[:], tmp[:], MAGIC)
                            nc.vector.tensor_scalar_sub(tmp[:], tmp[:], MAGIC)
                            nc.vector.scalar_tensor_tensor(
                                red[:], tmp[:], -TWOPI, ang[:],
                                op0=mul_op, op1=add_op)
                            if phase != 0.0:
                                nc.vector.tensor_scalar_add(red[:], red[:],
                                                            phase)
                            nc.vector.tensor_scalar_min(red[:], red[:],
                                                        3.141592)
                            nc.vector.tensor_scalar_max(red[:], red[:],
                                                        -3.141592)
                            nc.scalar.activation(dst[:, q0:q0 + 512], red[:],
                                                 Sin)

                # preload the Exp table set while A0 runs (saves the ~2.7us
                # table switch from the A1 critical path)
                dume = rp.tile([1, 2], F32, name="dume", tag="dume")
                nc.vector.memset(dume[:], 0.0)
                nc.scalar.activation(dume[:], dume[:], Exp)

                for c in range(NCH):
                    c0 = CHUNK * c
                    hidT = [hTp.tile([128, CHUNK], BF16,
                                     name=f"hidT{h}", tag=f"hidT{h}")
                            for h in range(NHT)]
                    for h in range(NHT):
                        if c == 0 and h % 8 == 0:
                            for hh in range(h, h + 8):
                                nc.sync.dma_start(
                                    wq_flat[:, hh, :],
                                    w_qkv[128 * hh:128 * (hh + 1), :])
                        nc.sync.dma_start_transpose(
                            hidT[h][:], hidden[c0:c0 + CHUNK,
                                               128 * h:128 * (h + 1)])
                    # last chunk: emit k cts first so the A1 kT/qT loads can
                    # start before the rest of the chunk finishes
                    ct_order = ([NQ, NQ + 1] + list(range(NQ))
                                if c == NCH - 1 else list(range(NQK)))
                    for ct in ct_order:
                        pq = psA.tile([128, CHUNK], F32, name="pq", tag="pq")
                        for h in range(NHT):
                            nc.tensor.matmul(
                                pq[:],
                                wq_sb[h][:, 128 * ct:128 * (ct + 1)],
                                hidT[h][:],
                                start=(h == 0), stop=(h == NHT - 1),
                            )
                        qw = rp.tile([128, CHUNK], RDT, name="qw", tag="qw")
                        nc.scalar.copy(qw[:], pq[:])
                        pswap = psW.tile([128, CHUNK], F32, name="pswap",
                                         tag="pswap")
                        nc.tensor.matmul(pswap[:], swap_sb[:], qw[:],
                                         start=True, stop=True)
                        qcos = rp.tile([128, CHUNK], BF16, name="qcos",
                                       tag="qcos")
                        nc.vector.tensor_mul(qcos[:], qw[:],
                                             cosc[:, c0:c0 + CHUNK])
                        rot = rp.tile([128, CHUNK], BF16, name="rot", tag="rot")
                        nc.vector.tensor_mul(rot[:], pswap[:],
                                             sinc[:, c0:c0 + CHUNK])
                        qout = rp.tile([128, CHUNK], BF16, name="qout",
                                       tag="qout")
                        nc.vector.tensor_add(qout[:], qcos[:], rot[:])
                        nc.sync.dma_start(qkT[ct][:, c0:c0 + CHUNK], qout[:])
                    for tt in range(CHUNK // 128):
                        pv = psV.tile([128, KC], F32, name="pv", tag="pv")
                        for h in range(NHT):
                            nc.tensor.matmul(
                                pv[:],
                                hidT[h][:, 128 * tt:128 * (tt + 1)],
                                wq_sb[h][:, QC + KC:QC + 2 * KC],
                                start=(h == 0), stop=(h == NHT - 1),
                            )
                        vw = vwp.tile([128, KC], BF16, name="vw", tag="vw")
                        nc.scalar.copy(vw[:], pv[:])
                        nc.sync.dma_start(v_dr[c * (CHUNK // 128) + tt], vw[:])

            # ============ A1: attention + o-proj + RS ============
            with (
                tc.tile_pool(name="qT", bufs=1) as qTp,
                tc.tile_pool(name="kT", bufs=1) as kTp,
                tc.tile_pool(name="vsb", bufs=1) as vp,
                tc.tile_pool(name="wo", bufs=1) as wop,
                tc.tile_pool(name="maskp", bufs=1) as mkp,
                tc.tile_pool(name="at", bufs=2) as atp,
                tc.tile_pool(name="den", bufs=2) as dnp,
                tc.tile_pool(name="pt", bufs=3) as ptp,
                tc.tile_pool(name="bcp", bufs=2) as bcp,
                tc.tile_pool(name="ot", bufs=3) as otp,
                tc.tile_pool(name="psS", bufs=2, space="PSUM") as psS,
                tc.tile_pool(name="psPV", bufs=2, space="PSUM") as psPV,
                tc.tile_pool(name="psX", bufs=2, space="PSUM") as psX,
            ):
                psO = psX
                kT_sb = [kTp.tile([128, S], BF16, name=f"kT{i}", tag=f"kT{i}")
                         for i in range(NKV)]
                v_flat = vp.tile([128, NTT, KC], BF16, name="vfl", tag="vfl")
                v_sb = [v_flat[:, t, :] for t in range(NTT)]
                qT_sb = [qTp.tile([128, S], BF16, name=f"qT{i}", tag=f"qT{i}")
                         for i in range(NQ)]
                mask_sb = mkp.tile([128, SKT, SLAB], BF16, name="masks",
                                   tag="masks")
                wo_flat = wop.tile([128, NQ, HID], BF16, name="wofl", tag="wofl")
                wo_sb = [wo_flat[:, q, :] for q in range(NQ)]
                # load order tuned so slab-0/head-0 can start ASAP
                for i in range(NKV):
                    nc.sync.dma_start(kT_sb[i][:], qkT[NQ + i])
                nc.sync.dma_start(qT_sb[0][:], qkT[0])
                nc.sync.dma_start(mask_sb[:], masks[:])
                for t in range(SKT):
                    nc.sync.dma_start(v_flat[:, t, :], v_dr[t])
                for i in range(1, NQ):
                    nc.sync.dma_start(qT_sb[i][:], qkT[i])
                for t in range(SKT, NTT):
                    nc.sync.dma_start(v_flat[:, t, :], v_dr[t])
                for q in range(NQ):
                    nc.sync.dma_start(wo_flat[:, q, :],
                                      w_o[128 * q:128 * (q + 1), :])

                def oproj_piece(s, pi, ats):
                    tt, hc = divmod(pi, HID // 512)
                    po = psO.tile([128, 512], F32, name="po", tag="px")
                    for q in range(NQ):
                        nc.tensor.matmul(
                            po[:],
                            ats[q][:, 128 * tt:128 * (tt + 1)],
                            wo_sb[q][:, 512 * hc:512 * (hc + 1)],
                            start=(q == 0), stop=(q == NQ - 1),
                        )
                    ot = otp.tile([128, 512], BF16, name="ot", tag="ot")
                    if pi % 2 == 0:
                        nc.scalar.copy(ot[:], po[:])
                    else:
                        nc.vector.tensor_copy(ot[:], po[:])
                    if s < NSLAB - 1:
                        nc.sync.dma_start(
                            parts[s][128 * tt:128 * (tt + 1),
                                     512 * hc:512 * (hc + 1)], ot[:])
                        if pi == 4 * NQ - 1:
                            nc.gpsimd.collective_compute(
                                "ReduceScatter", mybir.AluOpType.add,
                                replica_groups=RG,
                                ins=[parts[s].opt()], outs=[rsouts[s].opt()])
                            nc.gpsimd.dma_start(out[s], rsouts[s][:])
                    else:
                        half, tt2 = divmod(tt, 2)
                        nc.sync.dma_start(
                            parts3[half][128 * tt2:128 * (tt2 + 1),
                                         512 * hc:512 * (hc + 1)], ot[:])
                        if pi in (2 * NQ - 1, 4 * NQ - 1):
                            half = pi // (2 * NQ)
                            nc.gpsimd.collective_compute(
                                "ReduceScatter", mybir.AluOpType.add,
                                replica_groups=RG,
                                ins=[parts3[half].opt()],
                                outs=[rsouts3[half].opt()])
                            nc.gpsimd.dma_start(
                                out[NSLAB - 1][64 * half:64 * (half + 1), :],
                                rsouts3[half][:])

                at_prev = None
                for s in range(NSLAB):
                    s0 = SLAB * s
                    NSUP = 2 * (s + 1)
                    sup_total = NQ * NSUP
                    sup_done = 0
                    pieces_emitted = 0
                    at_cur = []
                    for hq in range(NQ):
                        kvh = hq // REP
                        ppv = psPV.tile([128, SLAB], F32, name="ppv", tag="ppv")
                        den = dnp.tile([128, 2, 512], BF16, name="den",
                                       tag="den")
                        den_src = None
                        for j in range(NSUP):
                            ps = psS.tile([128, 2, 512], F32, name="ps",
                                          tag="ps")
                            for u in range(2):
                                kt = 2 * j + u
                                nc.tensor.matmul(
                                    ps[:, u, :],
                                    kT_sb[kvh][:, 128 * kt:128 * (kt + 1)],
                                    qT_sb[hq][:, s0:s0 + SLAB],
                                    start=True, stop=True,
                                )
                            pt = ptp.tile([128, 2, 512], BF16, name="pt",
                                          tag="pt")
                            nc.scalar.activation(pt[:], ps[:], Exp, scale=SCALE)
                            dj = j - 2 * s
                            if 0 <= dj <= 1:
                                nc.vector.tensor_mul(
                                    pt[:], pt[:],
                                    mask_sb[:, 2 * dj:2 * dj + 2, :])
                            if NSUP > 1:
                                if j == 0:
                                    nc.vector.tensor_copy(den[:], pt[:])
                                else:
                                    nc.vector.tensor_add(den[:], den[:], pt[:])
                                den_src = den
                            else:
                                den_src = pt
                            # fill the QK->exp->PV latency with o-proj MMs of
                            # the previous slab (keeps the PE dense and warm)
                            sup_done += 1
                            if at_prev is not None:
                                due = (4 * NQ * sup_done) // sup_total
                                while pieces_emitted < due:
                                    oproj_piece(s - 1, pieces_emitted, at_prev)
                                    pieces_emitted += 1
                            for u in range(2):
                                kt = 2 * j + u
                                nc.tensor.matmul(
                                    ppv[:],
                                    v_sb[kt][:, D * kvh:D * (kvh + 1)],
                                    pt[:, u, :],
                                    start=(j == 0 and u == 0),
                                    stop=(j == NSUP - 1 and u == 1),
                                )
                        pden = psX.tile([128, 512], F32, name="pden",
                                        tag="px")
                        for u in range(2):
                            nc.tensor.matmul(pden[0:1, :], ones_col[:],
                                             den_src[:, u, :],
                                             start=(u == 0), stop=(u == 1))
                        rec = bcp.tile([1, 512], F32, name="rec", tag="rec")
                        nc.vector.reciprocal_approx_fast(rec[:], pden[0:1, :])
                        pbc = psX.tile([128, 512], F32, name="pbc", tag="px")
                        nc.tensor.matmul(pbc[:], ones_row[:], rec[:],
                                         start=True, stop=True)
                        bc = bcp.tile([128, 512], F32, name="bc", tag="bc")
                        nc.vector.tensor_copy(bc[:], pbc[:])
                        at = atp.tile([128, SLAB], BF16,
                                      name=f"at{hq}", tag=f"at{hq}")
                        nc.vector.tensor_mul(at[:], ppv[:], bc[:])
                        at_cur.append(at)
                    while at_prev is not None and pieces_emitted < 4 * NQ:
                        oproj_piece(s - 1, pieces_emitted, at_prev)
                        pieces_emitted += 1
                    at_prev = at_cur
                for pi in range(4 * NQ):
                    oproj_piece(NSLAB - 1, pi, at_prev)

    nc.compile()
    return nc


def make_consts(S=2048):
    SLAB = min(512, S)
    SKT = SLAB // 128
    d_half = np.arange(0, D, 2, dtype=np.float32) / D
    invfreq = (1.0 / (ROPE_THETA ** d_half)).astype(np.float32)  # [64]
    # signed: rows 0..63 negative (sin sign trick), cos unaffected (even fn)
    invf128 = np.concatenate([-invfreq, invfreq]).reshape(128, 1).astype(np.float32)
    p = np.arange(128).reshape(128, 1, 1)
    j = np.arange(SKT).reshape(1, SKT, 1)
    q = np.arange(SLAB).reshape(1, 1, SLAB)
    masks = ((j * 128 + p) <= q).astype(ml_dtypes.bfloat16)  # [128, SKT, SLAB]
    swapmat = np.zeros((128, 128), np.float32)
    for pp in range(128):
        swapmat[pp, (pp + 64) % 128] = 1.0
    return invf128, masks, swapmat.astype(ml_dtypes.bfloat16)


def shard_inputs(hidden_states, positions, w_qkv, w_o, n_q_total=32, n_kv_total=8,
                 tp=4):
    """Returns in_maps for 8 cores: DP over batch x TP over heads."""
    B, S, HIDDEN = hidden_states.shape
    q_size = n_q_total * D
    kv_size = n_kv_total * D
    nq = n_q_total // tp           # q heads per core
    nkv = n_kv_total // tp         # kv heads per core
    invf128, masks, swapmat = make_consts(S=S)
    in_maps = []
    for c in range(8):
        g, r = divmod(c, tp)
        wq = w_qkv[:, nq * D * r: nq * D * (r + 1)]
        wk = w_qkv[:, q_size + nkv * D * r: q_size + nkv * D * (r + 1)]
        wv = w_qkv[:, q_size + kv_size + nkv * D * r: q_size + kv_size + nkv * D * (r + 1)]
        in_maps.append({
            "hidden_states": np.ascontiguousarray(
                hidden_states[g]).astype(ml_dtypes.bfloat16),
            "positions": np.ascontiguousarray(positions[g:g + 1]).astype(np.int32),
            "w_qkv": np.ascontiguousarray(
                np.concatenate([wq, wk, wv], axis=1)).astype(ml_dtypes.bfloat16),
            "w_o": np.ascontiguousarray(
                w_o[nq * D * r: nq * D * (r + 1), :]).astype(ml_dtypes.bfloat16),
            "invfreq128": invf128,
            "masks": masks,
            "swapmat": swapmat,
        })
    return in_maps


def assemble_output(results, B=2, S=2048, HIDDEN=4096, tp=4):
    SLAB = min(512, S)
    NSLAB = S // SLAB
    out = np.empty((B, S, HIDDEN), dtype=np.float32)
    for c in range(8):
        g, r = divmod(c, tp)
        o = np.asarray(results[c]["out"]).reshape(NSLAB, SLAB // 4, HIDDEN)
        for s in range(NSLAB - 1):
            t0 = SLAB * s + 128 * r
            out[g, t0:t0 + 128, :] = o[s]
        for half in range(2):
            t0 = SLAB * (NSLAB - 1) + 256 * half + 64 * r
            out[g, t0:t0 + 64, :] = o[NSLAB - 1][64 * half:64 * (half + 1)]
    return out


def kernel(hidden_states, positions, w_qkv, w_o):
    hidden_states = np.asarray(hidden_states, dtype=np.float32)
    positions = np.asarray(positions, dtype=np.int32)
    w_qkv = np.asarray(w_qkv, dtype=np.float32)
    w_o = np.asarray(w_o, dtype=np.float32)
    B, S, HIDDEN = hidden_states.shape

    key = (S, HIDDEN)
    if key not in _NC_CACHE:
        try:
            _NC_CACHE[key] = build_nc(S=S, HID=HIDDEN, rope_f32r=True)
        except Exception:
            _NC_CACHE[key] = build_nc(S=S, HID=HIDDEN, rope_f32r=False)
    nc = _NC_CACHE[key]

    in_maps = shard_inputs(hidden_states, positions, w_qkv, w_o)
    res = run_bass_kernel_spmd(nc, in_maps, core_ids=list(range(8)))
    return assemble_output(res.results, B=B, S=S, HIDDEN=HIDDEN)


if __name__ == "__main__":
    rng = np.random.default_rng(0)
    B, S, HIDDEN = 2, 2048, 4096
    hs = rng.standard_normal((B, S, HIDDEN), dtype=np.float32)
    pos = np.arange(B * S, dtype=np.int32).reshape(B, S)
    wq = rng.standard_normal((HIDDEN, 6144), dtype=np.float32) * HIDDEN ** -0.5
    wo = rng.standard_normal((4096, HIDDEN), dtype=np.float32) * 4096 ** -0.5
    o = kernel(hs, pos, wq, wo)
    print(o.shape, o.dtype)


# revision 44
# speedup vs baseline: 1.0824x; 1.0126x over previous
"""Distributed Trainium2 kernel for ArceeAttention (GQA + RoPE + causal attention).

Sharding: DP over batch (2 groups of 4 cores) x TP-4 over heads within each group.
Each core: 8 q heads + 2 kv heads, full sequence of its batch.

Structure (engine-dense rework of the baseline; 1452us -> ~1170us):
  - Inputs are pre-cast to bf16 on the HOST in shard_inputs (halves all
    weight/hidden HBM traffic; kills the on-device staging casts entirely).
  - A0: chunk-major QKV with N=512 moving operand; hidT via xbar transposes
    [512,128] read straight from the bf16 hidden input.  ALL A0 DMA rides the
    single sync HWDGE ring: any DMA concurrent with an in-flight xbar
    transpose is starved by the transpose's exclusive hold on the DMA engines
    (and transposes on TWO HWDGE rings concurrently corrupt data).  Weight
    loads are interleaved with the chunk-0 transposes in 8-tile batches.
    RoPE trig precomputed once (quartered scratch so SBUF stays disjoint from
    the weight region -- no WAR serialization); sin sign folded into invfreq
    (signed angle).  qT/kT/v round-trip via DRAM to fit SBUF.
  - A1: scores in [128, 2x512] PSUM supertiles, ONE exp per supertile,
    denominator accumulated on DVE in bf16 and folded by ones-matmuls;
    o-proj pieces of slab s-1 interleaved between QK and PV inside slab s's
    supertile loop (fills the QK->exp->PV latency, keeps PE dense and the
    HAM clock warm); po/pden/pbc share a 2-buf PSUM pool; o-proj evacs
    alternate ACT/DVE.  ReduceScatter per full slab for slabs 0-2 and in two
    halves for slab 3 so the kernel tail only waits for a 2MB collective.
"""
import sys
import numpy as np

for _p in ("/opt/trn_rl_repo",):
    if _p not in sys.path:
        sys.path.append(_p)

import ml_dtypes  # noqa: E402
from concourse import bass, bacc, tile, mybir  # noqa: E402
from concourse.bass_utils import run_bass_kernel_spmd  # noqa: E402

F32 = mybir.dt.float32
F32R = mybir.dt.float32r
BF16 = mybir.dt.bfloat16
I32 = mybir.dt.int32

ROPE_THETA = 10000.0
D = 128  # head dim

_NC_CACHE = {}


def build_nc(S=2048, HID=4096, NQ=8, NKV=2, rope_f32r=True):
    REP = NQ // NKV           # q heads per kv head (4)
    QC = NQ * D               # q cols per core (1024)
    KC = NKV * D              # k (or v) cols per core (256)
    NQK = NQ + NKV            # q+k col-tiles (10)
    NHT = HID // 128          # hidden-dim tiles (32)
    CHUNK = 512
    NCH = S // CHUNK          # 4
    SLAB = 512
    NSLAB = S // SLAB         # 4
    SKT = SLAB // 128         # 4
    NTT = S // 128            # 16 token tiles
    SCALE = float(D) ** -0.5
    RG = [[0, 1, 2, 3], [4, 5, 6, 7]]
    RDT = BF16

    MAGIC = 12582912.0        # 1.5 * 2**23: float32 round-to-nearest-int trick
    TWOPI = float(2.0 * np.pi)
    INV2PI = float(1.0 / TWOPI)
    HALFPI = float(np.pi / 2.0)

    nc = bacc.Bacc(None, target_bir_lowering=False)
    hidden = nc.declare_dram_parameter("hidden_states", [S, HID], BF16, isOutput=False)
    positions = nc.declare_dram_parameter("positions", [1, S], I32, isOutput=False)
    w_qkv = nc.declare_dram_parameter("w_qkv", [HID, QC + 2 * KC], BF16, isOutput=False)
    w_o = nc.declare_dram_parameter("w_o", [QC, HID], BF16, isOutput=False)
    invf = nc.declare_dram_parameter("invfreq128", [128, 1], F32, isOutput=False)
    masks = nc.declare_dram_parameter("masks", [128, SKT, SLAB], BF16, isOutput=False)
    swapm = nc.declare_dram_parameter("swapmat", [128, 128], RDT, isOutput=False)
    # slabs 0..2: rows = this core's 128-token quarter of the slab;
    # slab 3: rows 0:64 = quarter of half-slab 3a, rows 64:128 = 3b.
    out = nc.declare_dram_parameter("out", [NSLAB, SLAB // 4, HID], F32,
                                    isOutput=True)

    Exp = mybir.ActivationFunctionType.Exp
    Sin = mybir.ActivationFunctionType.Sin
    mul_op = mybir.AluOpType.mult
    add_op = mybir.AluOpType.add

    with tile.TileContext(nc) as tc:
      with tc.tile_pool(name="dram", bufs=1, space="DRAM") as dram:
        qkT = dram.tile([NQK, 128, S], BF16, name="qkT", tag="qkT")
        v_dr = dram.tile([NTT, 128, KC], BF16, name="v_dr", tag="v_dr")
        # slabs 0..2: one RS over the full slab; slab 3: two half-slab RS so
        # the kernel tail only waits for a 2MB collective.
        parts = [dram.tile([SLAB, HID], BF16, name=f"part{i}", tag=f"part{i}")
                 for i in range(NSLAB - 1)]
        parts3 = [dram.tile([SLAB // 2, HID], BF16, name=f"part3{i}",
                            tag=f"part3{i}") for i in range(2)]
        rsouts = [dram.tile([SLAB // 4, HID], BF16, name=f"rsout{i}",
                            tag=f"rsout{i}")
                  for i in range(NSLAB - 1)]
        rsouts3 = [dram.tile([SLAB // 8, HID], BF16, name=f"rsout3{i}",
                             tag=f"rsout3{i}") for i in range(2)]

        with tc.tile_pool(name="const", bufs=1) as cpool:
            invf_sb = cpool.tile([128, 1], F32, name="invf", tag="invf")
            nc.sync.dma_start(invf_sb[:], invf[:])
            ones_col = cpool.tile([128, 1], BF16, name="ones_col", tag="ones_col")
            nc.vector.memset(ones_col[:], 1.0)
            ones_row = cpool.tile([1, 128], F32, name="ones_row", tag="ones_row")
            nc.vector.memset(ones_row[:], 1.0)
            swap_sb = cpool.tile([128, 128], RDT, name="swapm", tag="swapm")
            nc.sync.dma_start(swap_sb[:], swapm[:])
            cosc = cpool.tile([128, S], BF16, name="cosc", tag="cosc")
            sinc = cpool.tile([128, S], BF16, name="sinc", tag="sinc")

            # ================= A0: QKV + RoPE =================
            with (
                tc.tile_pool(name="wq", bufs=1) as wqp,
                tc.tile_pool(name="hidT", bufs=2) as hTp,
                tc.tile_pool(name="rope", bufs=3) as rp,
                tc.tile_pool(name="vw", bufs=2) as vwp,
                tc.tile_pool(name="psA", bufs=3, space="PSUM") as psA,
                tc.tile_pool(name="psW", bufs=2, space="PSUM") as psW,
                tc.tile_pool(name="psV", bufs=2, space="PSUM") as psV,
            ):
                WCOLS = QC + 2 * KC
                wq_flat = wqp.tile([128, NHT, WCOLS], BF16, name="wq", tag="wq")
                wq_sb = [wq_flat[:, h, :] for h in range(NHT)]
                # All A0 DMA goes on the ONE sync HWDGE ring: any DMA that
                # runs concurrently with an in-flight xbar transpose is
                # starved by the transpose's exclusive hold on the DMA
                # engines, so serialize everything on one ring instead.
                # (Weight loads are interleaved with the chunk-0 transposes
                # in batches of 8 below.)

                # ---- trig precompute (quartered; small disjoint scratch) ----
                with (
                    tc.tile_pool(name="trig", bufs=1) as tgp,
                    tc.tile_pool(name="psTR", bufs=1, space="PSUM") as ptr,
                ):
                    for qi in range(S // 512):
                        q0 = 512 * qi
                        pos_i = tgp.tile([1, 512], I32, name="posi", tag="posi")
                        nc.sync.dma_start(pos_i[:], positions[0:1, q0:q0 + 512])
                        pos_c = tgp.tile([1, 512], F32, name="posc", tag="posc")
                        nc.vector.tensor_copy(pos_c[:], pos_i[:])
                        ppos = ptr.tile([128, 512], F32, name="ppos", tag="ppos")
                        nc.tensor.matmul(ppos[:], ones_row[:], pos_c[:],
                                         start=True, stop=True)
                        # signed angle: invf rows 0..63 negative -> sin gets
                        # the sign for free, cos is even so unaffected.
                        ang = tgp.tile([128, 512], F32, name="ang", tag="ang")
                        nc.vector.tensor_scalar_mul(ang[:], ppos[:], invf_sb[:])
                        tmp = tgp.tile([128, 512], F32, name="ttmp", tag="ttmp")
                        red = tgp.tile([128, 512], F32, name="tred", tag="tred")
                        for dst, phase in ((cosc, HALFPI), (sinc, 0.0)):
                            nc.vector.tensor_scalar(
                                tmp[:], ang[:], INV2PI, phase * INV2PI,
                                op0=mul_op, op1=add_op)
                            nc.vector.tensor_scalar_add(tmp[:], tmp[:], MAGIC)
                            nc.vector.tensor_scalar_sub(tmp[:], tmp[:], MAGIC)
                            nc.vector.scalar_tensor_tensor(
                                red[:], tmp[:], -TWOPI, ang[:],
                                op0=mul_op, op1=add_op)
                            if phase != 0.0:
                                nc.vector.tensor_scalar_add(red[:], red[:],
                                                            phase)
                            nc.vector.tensor_scalar_min(red[:], red[:],
                                                        3.141592)
                            nc.vector.tensor_scalar_max(red[:], red[:],
                                                        -3.141592)
                            nc.scalar.activation(dst[:, q0:q0 + 512], red[:],
                                                 Sin)

                # preload the Exp table set while A0 runs (saves the ~2.7us
                # table switch from the A1 critical path)
                dume = rp.tile([1, 2], F32, name="dume", tag="dume")
                nc.vector.memset(dume[:], 0.0)
                nc.scalar.activation(dume[:], dume[:], Exp)

                for c in range(NCH):
                    c0 = CHUNK * c
                    hidT = [hTp.tile([128, CHUNK], BF16,
                                     name=f"hidT{h}", tag=f"hidT{h}")
                            for h in range(NHT)]
                    for h in range(NHT):
                        if c == 0 and h % 8 == 0:
                            for hh in range(h, h + 8):
                                nc.sync.dma_start(
                                    wq_flat[:, hh, :],
                                    w_qkv[128 * hh:128 * (hh + 1), :])
                        nc.sync.dma_start_transpose(
                            hidT[h][:], hidden[c0:c0 + CHUNK,
                                               128 * h:128 * (h + 1)])
                    # last chunk: emit k cts first so the A1 kT/qT loads can
                    # start before the rest of the chunk finishes
                    ct_order = ([NQ, NQ + 1] + list(range(NQ))
                                if c == NCH - 1 else list(range(NQK)))
                    for ct in ct_order:
                        pq = psA.tile([128, CHUNK], F32, name="pq", tag="pq")
                        for h in range(NHT):
                            nc.tensor.matmul(
                                pq[:],
                                wq_sb[h][:, 128 * ct:128 * (ct + 1)],
                                hidT[h][:],
                                start=(h == 0), stop=(h == NHT - 1),
                            )
                        qw = rp.tile([128, CHUNK], RDT, name="qw", tag="qw")
                        nc.scalar.copy(qw[:], pq[:])
                        pswap = psW.tile([128, CHUNK], F32, name="pswap",
                                         tag="pswap")
                        nc.tensor.matmul(pswap[:], swap_sb[:], qw[:],
                                         start=True, stop=True)
                        qcos = rp.tile([128, CHUNK], BF16, name="qcos",
                                       tag="qcos")
                        nc.vector.tensor_mul(qcos[:], qw[:],
                                             cosc[:, c0:c0 + CHUNK])
                        rot = rp.tile([128, CHUNK], BF16, name="rot", tag="rot")
                        nc.vector.tensor_mul(rot[:], pswap[:],
                                             sinc[:, c0:c0 + CHUNK])
                        qout = rp.tile([128, CHUNK], BF16, name="qout",
                                       tag="qout")
                        nc.vector.tensor_add(qout[:], qcos[:], rot[:])
                        nc.sync.dma_start(qkT[ct][:, c0:c0 + CHUNK], qout[:])
                    for tt in range(CHUNK // 128):
                        pv = psV.tile([128, KC], F32, name="pv", tag="pv")
                        for h in range(NHT):
                            nc.tensor.matmul(
                                pv[:],
                                hidT[h][:, 128 * tt:128 * (tt + 1)],
                                wq_sb[h][:, QC + KC:QC + 2 * KC],
                                start=(h == 0), stop=(h == NHT - 1),
                            )
                        vw = vwp.tile([128, KC], BF16, name="vw", tag="vw")
                        nc.scalar.copy(vw[:], pv[:])
                        nc.sync.dma_start(v_dr[c * (CHUNK // 128) + tt], vw[:])

            # ============ A1: attention + o-proj + RS ============
            with (
                tc.tile_pool(name="qT", bufs=1) as qTp,
                tc.tile_pool(name="kT", bufs=1) as kTp,
                tc.tile_pool(name="vsb", bufs=1) as vp,
                tc.tile_pool(name="wo", bufs=1) as wop,
                tc.tile_pool(name="maskp", bufs=1) as mkp,
                tc.tile_pool(name="at", bufs=2) as atp,
                tc.tile_pool(name="den", bufs=2) as dnp,
                tc.tile_pool(name="pt", bufs=5) as ptp,
                tc.tile_pool(name="bcp", bufs=2) as bcp,
                tc.tile_pool(name="ot", bufs=4) as otp,
                tc.tile_pool(name="psS", bufs=2, space="PSUM") as psS,
                tc.tile_pool(name="psPV", bufs=2, space="PSUM") as psPV,
                tc.tile_pool(name="psX", bufs=2, space="PSUM") as psX,
            ):
                psO = psX
                kT_sb = [kTp.tile([128, S], BF16, name=f"kT{i}", tag=f"kT{i}")
                         for i in range(NKV)]
                v_flat = vp.tile([128, NTT, KC], BF16, name="vfl", tag="vfl")
                v_sb = [v_flat[:, t, :] for t in range(NTT)]
                qT_sb = [qTp.tile([128, S], BF16, name=f"qT{i}", tag=f"qT{i}")
                         for i in range(NQ)]
                mask_sb = mkp.tile([128, SKT, SLAB], BF16, name="masks",
                                   tag="masks")
                wo_flat = wop.tile([128, NQ, HID], BF16, name="wofl", tag="wofl")
                wo_sb = [wo_flat[:, q, :] for q in range(NQ)]
                # load order tuned so slab-0/head-0 can start ASAP
                for i in range(NKV):
                    nc.sync.dma_start(kT_sb[i][:], qkT[NQ + i])
                nc.sync.dma_start(qT_sb[0][:], qkT[0])
                nc.sync.dma_start(mask_sb[:], masks[:])
                for t in range(SKT):
                    nc.sync.dma_start(v_flat[:, t, :], v_dr[t])
                for i in range(1, NQ):
                    nc.sync.dma_start(qT_sb[i][:], qkT[i])
                for t in range(SKT, NTT):
                    nc.sync.dma_start(v_flat[:, t, :], v_dr[t])
                for q in range(NQ):
                    nc.sync.dma_start(wo_flat[:, q, :],
                                      w_o[128 * q:128 * (q + 1), :])

                def oproj_piece(s, pi, ats):
                    tt, hc = divmod(pi, HID // 512)
                    po = psO.tile([128, 512], F32, name="po", tag="px")
                    for q in range(NQ):
                        nc.tensor.matmul(
                            po[:],
                            ats[q][:, 128 * tt:128 * (tt + 1)],
                            wo_sb[q][:, 512 * hc:512 * (hc + 1)],
                            start=(q == 0), stop=(q == NQ - 1),
                        )
                    ot = otp.tile([128, 512], BF16, name="ot", tag="ot")
                    if pi % 2 == 0:
                        nc.scalar.copy(ot[:], po[:])
                    else:
                        nc.vector.tensor_copy(ot[:], po[:])
                    if s < NSLAB - 1:
                        nc.sync.dma_start(
                            parts[s][128 * tt:128 * (tt + 1),
                                     512 * hc:512 * (hc + 1)], ot[:])
                        if pi == 4 * NQ - 1:
                            nc.gpsimd.collective_compute(
                                "ReduceScatter", mybir.AluOpType.add,
                                replica_groups=RG,
                                ins=[parts[s].opt()], outs=[rsouts[s].opt()])
                            nc.gpsimd.dma_start(out[s], rsouts[s][:])
                    else:
                        half, tt2 = divmod(tt, 2)
                        nc.sync.dma_start(
                            parts3[half][128 * tt2:128 * (tt2 + 1),
                                         512 * hc:512 * (hc + 1)], ot[:])
                        if pi in (2 * NQ - 1, 4 * NQ - 1):
                            half = pi // (2 * NQ)
                            nc.gpsimd.collective_compute(
                                "ReduceScatter", mybir.AluOpType.add,
                                replica_groups=RG,
                                ins=[parts3[half].opt()],
                                outs=[rsouts3[half].opt()])
                            nc.gpsimd.dma_start(
                                out[NSLAB - 1][64 * half:64 * (half + 1), :],
                                rsouts3[half][:])

                at_prev = None
                for s in range(NSLAB):
                    s0 = SLAB * s
                    NSUP = 2 * (s + 1)
                    sup_total = NQ * NSUP
                    sup_done = 0
                    pieces_emitted = 0
                    at_cur = []
                    for hq in range(NQ):
                        kvh = hq // REP
                        ppv = psPV.tile([128, SLAB], F32, name="ppv", tag="ppv")
                        den = dnp.tile([128, 2, 512], BF16, name="den",
                                       tag="den")
                        den_src = None
                        for j in range(NSUP):
                            ps = psS.tile([128, 2, 512], F32, name="ps",
                                          tag="ps")
                            for u in range(2):
                                kt = 2 * j + u
                                nc.tensor.matmul(
                                    ps[:, u, :],
                                    kT_sb[kvh][:, 128 * kt:128 * (kt + 1)],
                                    qT_sb[hq][:, s0:s0 + SLAB],
                                    start=True, stop=True,
                                )
                            pt = ptp.tile([128, 2, 512], BF16, name="pt",
                                          tag="pt")
                            nc.scalar.activation(pt[:], ps[:], Exp, scale=SCALE)
                            dj = j - 2 * s
                            if 0 <= dj <= 1:
                                nc.vector.tensor_mul(
                                    pt[:], pt[:],
                                    mask_sb[:, 2 * dj:2 * dj + 2, :])
                            if NSUP > 1:
                                if j == 0:
                                    nc.vector.tensor_copy(den[:], pt[:])
                                else:
                                    nc.vector.tensor_add(den[:], den[:], pt[:])
                                den_src = den
                            else:
                                den_src = pt
                            # fill the QK->exp->PV latency with o-proj MMs of
                            # the previous slab (keeps the PE dense and warm)
                            sup_done += 1
                            if at_prev is not None:
                                due = (4 * NQ * sup_done) // sup_total
                                while pieces_emitted < due:
                                    oproj_piece(s - 1, pieces_emitted, at_prev)
                                    pieces_emitted += 1
                            for u in range(2):
                                kt = 2 * j + u
                                nc.tensor.matmul(
                                    ppv[:],
                                    v_sb[kt][:, D * kvh:D * (kvh + 1)],
                                    pt[:, u, :],
                                    start=(j == 0 and u == 0),
                                    stop=(j == NSUP - 1 and u == 1),
                                )
                        pden = psX.tile([128, 512], F32, name="pden",
                                        tag="px")
                        for u in range(2):
                            nc.tensor.matmul(pden[0:1, :], ones_col[:],
                                             den_src[:, u, :],
                                             start=(u == 0), stop=(u == 1))
                        rec = bcp.tile([1, 512], F32, name="rec", tag="rec")
                        nc.vector.reciprocal_approx_fast(rec[:], pden[0:1, :])
                        pbc = psX.tile([128, 512], F32, name="pbc", tag="px")
                        nc.tensor.matmul(pbc[:], ones_row[:], rec[:],
                                         start=True, stop=True)
                        bc = bcp.tile([128, 512], F32, name="bc", tag="bc")
                        nc.vector.tensor_copy(bc[:], pbc[:])
                        at = atp.tile([128, SLAB], BF16,
                                      name=f"at{hq}", tag=f"at{hq}")
                        nc.vector.tensor_mul(at[:], ppv[:], bc[:])
                        at_cur.append(at)
                    while at_prev is not None and pieces_emitted < 4 * NQ:
                        oproj_piece(s - 1, pieces_emitted, at_prev)
                        pieces_emitted += 1
                    at_prev = at_cur
                for pi in range(4 * NQ):
                    oproj_piece(NSLAB - 1, pi, at_prev)

    nc.compile()
    return nc


def make_consts(S=2048):
    SLAB = min(512, S)
    SKT = SLAB // 128
    d_half = np.arange(0, D, 2, dtype=np.float32) / D
    invfreq = (1.0 / (ROPE_THETA ** d_half)).astype(np.float32)  # [64]
    # signed: rows 0..63 negative (sin sign trick), cos unaffected (even fn)
    invf128 = np.concatenate([-invfreq, invfreq]).reshape(128, 1).astype(np.float32)
    p = np.arange(128).reshape(128, 1, 1)
    j = np.arange(SKT).reshape(1, SKT, 1)
    q = np.arange(SLAB).reshape(1, 1, SLAB)
    masks = ((j * 128 + p) <= q).astype(ml_dtypes.bfloat16)  # [128, SKT, SLAB]
    swapmat = np.zeros((128, 128), np.float32)
    for pp in range(128):
        swapmat[pp, (pp + 64) % 128] = 1.0
    return invf128, masks, swapmat.astype(ml_dtypes.bfloat16)


def shard_inputs(hidden_states, positions, w_qkv, w_o, n_q_total=32, n_kv_total=8,
                 tp=4):
    """Returns in_maps for 8 cores: DP over batch x TP over heads."""
    B, S, HIDDEN = hidden_states.shape
    q_size = n_q_total * D
    kv_size = n_kv_total * D
    nq = n_q_total // tp           # q heads per core
    nkv = n_kv_total // tp         # kv heads per core
    invf128, masks, swapmat = make_consts(S=S)
    in_maps = []
    for c in range(8):
        g, r = divmod(c, tp)
        wq = w_qkv[:, nq * D * r: nq * D * (r + 1)]
        wk = w_qkv[:, q_size + nkv * D * r: q_size + nkv * D * (r + 1)]
        wv = w_qkv[:, q_size + kv_size + nkv * D * r: q_size + kv_size + nkv * D * (r + 1)]
        in_maps.append({
            "hidden_states": np.ascontiguousarray(
                hidden_states[g]).astype(ml_dtypes.bfloat16),
            "positions": np.ascontiguousarray(positions[g:g + 1]).astype(np.int32),
            "w_qkv": np.ascontiguousarray(
                np.concatenate([wq, wk, wv], axis=1)).astype(ml_dtypes.bfloat16),
            "w_o": np.ascontiguousarray(
                w_o[nq * D * r: nq * D * (r + 1), :]).astype(ml_dtypes.bfloat16),
            "invfreq128": invf128,
            "masks": masks,
            "swapmat": swapmat,
        })
    return in_maps


def assemble_output(results, B=2, S=2048, HIDDEN=4096, tp=4):
    SLAB = min(512, S)
    NSLAB = S // SLAB
    out = np.empty((B, S, HIDDEN), dtype=np.float32)
    for c in range(8):
        g, r = divmod(c, tp)
        o = np.asarray(results[c]["out"]).reshape(NSLAB, SLAB // 4, HIDDEN)
        for s in range(NSLAB - 1):
            t0 = SLAB * s + 128 * r
            out[g, t0:t0 + 128, :] = o[s]
        for half in range(2):
            t0 = SLAB * (NSLAB - 1) + 256 * half + 64 * r
            out[g, t0:t0 + 64, :] = o[NSLAB - 1][64 * half:64 * (half + 1)]
    return out


def kernel(hidden_states, positions, w_qkv, w_o):
    hidden_states = np.asarray(hidden_states, dtype=np.float32)
    positions = np.asarray(positions, dtype=np.int32)
    w_qkv = np.asarray(w_qkv, dtype=np.float32)
    w_o = np.asarray(w_o, dtype=np.float32)
    B, S, HIDDEN = hidden_states.shape

    key = (S, HIDDEN)
    if key not in _NC_CACHE:
        try:
            _NC_CACHE[key] = build_nc(S=S, HID=HIDDEN, rope_f32r=True)
        except Exception:
            _NC_CACHE[key] = build_nc(S=S, HID=HIDDEN, rope_f32r=False)
    nc = _NC_CACHE[key]

    in_maps = shard_inputs(hidden_states, positions, w_qkv, w_o)
    res = run_bass_kernel_spmd(nc, in_maps, core_ids=list(range(8)))
    return assemble_output(res.results, B=B, S=S, HIDDEN=HIDDEN)


if __name__ == "__main__":
    rng = np.random.default_rng(0)
    B, S, HIDDEN = 2, 2048, 4096
    hs = rng.standard_normal((B, S, HIDDEN), dtype=np.float32)
    pos = np.arange(B * S, dtype=np.int32).reshape(B, S)
    wq = rng.standard_normal((HIDDEN, 6144), dtype=np.float32) * HIDDEN ** -0.5
    wo = rng.standard_normal((4096, HIDDEN), dtype=np.float32) * 4096 ** -0.5
    o = kernel(hs, pos, wq, wo)
    print(o.shape, o.dtype)
